# revision 10
# baseline (speedup 1.0000x reference)
"""Trainium2 Bass kernel for AdaBiDiff GNN message passing.

Per-core computation (data parallel over batch B=8, one batch element per core):
  xt (12,1536) -> softmax over t -> p, logp (t-major)
  kl[i,j] = rowterm[i] - sum_t p[i,t] logp[j,t];  A = (kl < 0.5)
  u_fwd = (A @ xt.T) / rowsum(A);  u_bwd = (A.T @ xt.T) / colsum(A)
  x_flat[n, t*64+h] = relu(xt[t,n] W1[h] + (0.9 u_fwd + 2.1 u_bwd)[n,t] W2[h])
  two MLP blocks (BN folded into weights on host) -> out (12,1536) per core.

Implementation notes:
  - augmented-G: phat=[p;0..;rowterm-0.5], lhat=[logp;0..;-1] at partitions
    0..11 and 32 (32-alignment), so Ghat = 0.5-kl and A = (Ghat > 0).
  - phat/lhat duplicated at partitions 64..96 so the two Ghat orientations
    run row-packed (tile_position (0,0) vs (64,0)) concurrently on the PE.
  - A-orientation compare on DVE (is_gt -> 0/1); AT-orientation on ScalarE
    (Sign -> -1/0/1), with the sign-affine correction folded into the
    u_fwd scaling: yA=(yA'+Sx)/2, rs=(rs'+N)/2 -> uf=(yA'+Sx)/(rs'+N).
  - ones column in the transposed-x stationary produces row/col sums free.
  - both product accumulators share PSUM banks (partitions 0-32 and 64-96
    of the same tiles) -> 3 banks, letting Ghat tiles double-buffer.
  - x_flat build 4-way row-packed (inputs replicated at partitions
    0/32/64/96).
  - matmul dtype float32r (1 col/cycle); A/AT tiles and xtT in bf16.
"""

import os
import numpy as np

import concourse.bass as bass
import concourse.bacc as bacc
import concourse.tile as tile
import concourse.mybir as mybir

F32 = mybir.dt.float32
F32R = mybir.dt.float32r
BF16 = mybir.dt.bfloat16
AF = mybir.ActivationFunctionType
ALU = mybir.AluOpType

B, T, N, H, TH, HID2, TOUT = 8, 12, 1536, 64, 768, 128, 12
NT = N // 128
NC = N // 512
AUG = 32

_cache = {}


def _build_nc():
    nc = bacc.Bacc("TRN2", target_bir_lowering=False, debug=False)
    d = {}

    def dp(name, shape, dt=F32R, out=False):
        d[name] = nc.declare_dram_parameter(name, list(shape), dt, isOutput=out)

    dp("x", (T, N))
    dp("e1t", (T, TH)); dp("e2a", (T, TH)); dp("e2b", (T, TH))
    dp("ew1", (TH, HID2)); dp("ew2", (HID2, HID2)); dp("ew3", (HID2, H)); dp("eproj", (TH, H))
    dp("dw1", (H, HID2)); dp("dw2", (HID2, HID2)); dp("dw3", (HID2, TOUT)); dp("dproj", (H, TOUT))
    dp("eb1", (HID2, 1), F32); dp("eb2", (HID2, 1), F32); dp("ebe", (H, 1), F32)
    dp("db1", (HID2, 1), F32); dp("db2", (HID2, 1), F32); dp("dbd", (TOUT, 1), F32)
    dp("i12", (T, T))
    dp("out", (T, N), F32, out=True)

    with tile.TileContext(nc) as tc:
        _kernel_body(tc, d)
    nc.compile()
    return nc


def _kernel_body(tc, d):
    nc = tc.nc
    CS = [slice(c * 512, (c + 1) * 512) for c in range(NC)]

    with tc.tile_pool(name="w", bufs=1) as w, tc.tile_pool(name="sb", bufs=1) as sb:

        def wload(name, shape, dt=F32R, src=None):
            t = w.tile(list(shape), dt, name=name, tag=name)
            nc.sync.dma_start(out=t[:], in_=src if src is not None else d[name].ap())
            return t

        def stile(name, shape, dt=F32R):
            return sb.tile(list(shape), dt, name=name, tag=name)

        # ---- inputs / weights; x and x_flat operands replicated at 4 offsets ----
        xt4 = stile("xt4", (128, N))
        e1t4 = stile("e1t4", (128, TH))
        e2a4 = stile("e2a4", (128, TH))
        e2b4 = stile("e2b4", (128, TH))
        for g in range(4):
            nc.sync.dma_start(out=xt4[32 * g:32 * g + T, :], in_=d["x"].ap())
            nc.sync.dma_start(out=e1t4[32 * g:32 * g + T, :], in_=d["e1t"].ap())
            nc.sync.dma_start(out=e2a4[32 * g:32 * g + T, :], in_=d["e2a"].ap())
            nc.sync.dma_start(out=e2b4[32 * g:32 * g + T, :], in_=d["e2b"].ap())
        xt = xt4[0:T, :]

        ew1 = wload("ew1", (128, 6, HID2), src=d["ew1"].ap().rearrange("(a p) m -> p a m", p=128))
        eproj = wload("eproj", (128, 6, H), src=d["eproj"].ap().rearrange("(a p) m -> p a m", p=128))
        ew2 = wload("ew2", (HID2, HID2))
        ew3 = wload("ew3", (HID2, H))
        dw1 = wload("dw1", (H, HID2))
        dw2 = wload("dw2", (HID2, HID2))
        dw3 = wload("dw3", (HID2, TOUT))
        dproj = wload("dproj", (H, TOUT))
        eb1 = wload("eb1", (HID2, 1), F32)
        eb2 = wload("eb2", (HID2, 1), F32)
        ebe = wload("ebe", (H, 1), F32)
        db1 = wload("db1", (HID2, 1), F32)
        db2 = wload("db2", (HID2, 1), F32)
        dbd = wload("dbd", (TOUT, 1), F32)
        i12 = wload("i12", (T, T))

        ones12 = w.tile([T, 1], F32R, name="ones12", tag="ones12")
        nc.vector.memset(ones12[:].bitcast(F32), 1.0)
        ones1 = w.tile([1, T], F32R, name="ones1", tag="ones1")
        nc.vector.memset(ones1[:].bitcast(F32), 1.0)
        nhalf = w.tile([1, 1], F32, name="nhalf", tag="nhalf")
        nc.vector.memset(nhalf[:], -0.5)

        # =========== Stage A: softmax chain (t-major) ===========
        phat = stile("phat", (97, N))
        lhat = stile("lhat", (97, N))
        xtT = stile("xtT", (128, NT, AUG + 1), BF16)
        Sx = stile("Sx", (T, 1), F32)

        nc.gpsimd.memset(phat[0:AUG + 1, :].bitcast(F32), 0.0)
        nc.gpsimd.memset(lhat[0:AUG + 1, :].bitcast(F32), 0.0)
        nc.vector.memset(lhat[AUG:AUG + 1, :].bitcast(F32), -1.0)
        nc.gpsimd.memset(xtT[:], 0.0)

        nc.vector.tensor_reduce(Sx[:], xt, mybir.AxisListType.X, ALU.add)

        with tc.tile_pool(name="pa", bufs=2, space="PSUM") as pa, \
             tc.tile_pool(name="pat", bufs=1, space="PSUM") as pat:
            ex = stile("ex", (T, N))
            nc.scalar.activation(ex[:], xt, AF.Exp)

            ls = stile("ls", (1, N))
            for c in range(NC):
                psA = pa.tile([1, 512], F32, name="psA", tag="pa")
                nc.tensor.matmul(psA[:], ones12[:], ex[:, CS[c]], start=True, stop=True)
                nc.scalar.activation(ls[:, CS[c]], psA[:], AF.Ln)

            for c in range(NC):
                lsB = pa.tile([T, 512], F32, name="lsB", tag="pa")
                nc.tensor.matmul(lsB[:], ones1[:], ls[:, CS[c]], start=True, stop=True)
                nc.vector.tensor_tensor(lhat[0:T, CS[c]], xt[:, CS[c]], lsB[:], ALU.subtract)

            nc.scalar.activation(phat[0:T, :], lhat[0:T, :], AF.Exp)

            q = stile("q", (T, N))
            nc.vector.tensor_tensor(q[:], phat[0:T, :], lhat[0:T, :], ALU.mult)
            for c in range(NC):
                psR = pa.tile([1, 512], F32, name="psR", tag="pa")
                nc.tensor.matmul(psR[:], ones12[:], q[:, CS[c]], start=True, stop=True)
                nc.scalar.activation(phat[AUG:AUG + 1, CS[c]], psR[:], AF.Identity, bias=nhalf[:])

            # transposed x with ones column (bf16): xtT[p, j, t] = xt[t, 128j+p]
            psT = pat.tile([128, NT, T], F32, name="psT", tag="psT")
            for j in range(NT):
                nc.tensor.matmul(psT[:, j, :], xt[:, j * 128:(j + 1) * 128], i12[:],
                                 start=True, stop=True)
            nc.vector.tensor_copy(xtT[:, :, 0:T], psT[:])
            nc.vector.memset(xtT[:, :, AUG:AUG + 1], 1.0)

        # duplicate phat/lhat at partitions 64..96 for PE row-packing
        nc.sync.dma_start(out=phat[64:97, :], in_=phat[0:AUG + 1, :])
        nc.sync.dma_start(out=lhat[64:97, :], in_=lhat[0:AUG + 1, :])

        # =========== Stage B: Ghat, adjacency, products ===========
        uf4 = stile("uf4", (128, N))
        ub4 = stile("ub4", (128, N))

        with tc.tile_pool(name="pp", bufs=1, space="PSUM") as pp, \
             tc.tile_pool(name="pgg", bufs=2, space="PSUM") as pgg, \
             tc.tile_pool(name="pgt", bufs=2, space="PSUM") as pgt, \
             tc.tile_pool(name="ab", bufs=3) as ab:

            # shared-bank product accumulators: rows 0..32 = [yA';rs'] (sign),
            # rows 64..96 = [yAT;cs] (0/1)
            prod = [pp.tile([128, 512], F32, name=f"prod{c}", tag=f"prod{c}") for c in range(NC)]

            for i in range(NT):
                Ai = ab.tile([128, N], BF16, name="Ai", tag="Ai")
                ATi = ab.tile([128, N], BF16, name="ATi", tag="ATi")
                isl = slice(i * 128, (i + 1) * 128)
                for c in range(NC):
                    psG = pgg.tile([128, 512], F32, name="psG", tag="psG")
                    nc.tensor.matmul(psG[:], phat[0:AUG + 1, isl], lhat[0:AUG + 1, CS[c]],
                                     start=True, stop=True, tile_position=(0, 0))
                    nc.vector.tensor_scalar(Ai[:, CS[c]], psG[:], 0.0, None, ALU.is_gt)
                    psGT = pgt.tile([128, 512], F32, name="psGT", tag="psGT")
                    nc.tensor.matmul(psGT[:], lhat[64:97, isl], phat[64:97, CS[c]],
                                     start=True, stop=True, tile_position=(64, 0))
                    nc.scalar.sign(ATi[:, CS[c]], psGT[:])
                for c in range(NC):
                    nc.tensor.matmul(prod[c][0:AUG + 1, :], xtT[:, i, :], ATi[:, CS[c]],
                                     start=(i == 0), stop=(i == NT - 1),
                                     skip_group_check=True, tile_position=(0, 0))
                    nc.tensor.matmul(prod[c][64:97, :], xtT[:, i, :], Ai[:, CS[c]],
                                     start=(i == 0), stop=(i == NT - 1),
                                     skip_group_check=True, tile_position=(0, 64))

            # ===== Stage C: scale products =====
            # uf = (yA' + Sx) / (rs' + N)   [sign-corrected];  ub = yAT / cs
            rr = stile("rr", (1, N), F32R)
            cc = stile("cc", (1, N), F32R)
            tmpf = stile("tmpf", (1, N), F32)
            vf = stile("vf", (T, N), F32)
            vb = stile("vb", (T, N), F32)
            with nc.allow_low_precision(reason="4-byte recips"):
                for c in range(NC):
                    nc.vector.tensor_scalar(tmpf[:, CS[c]], prod[c][AUG:AUG + 1, :],
                                            float(N), None, ALU.add)
                    nc.vector.reciprocal(rr[:, CS[c]], tmpf[:, CS[c]])
                    nc.vector.reciprocal(cc[:, CS[c]], prod[c][96:97, :])
            for c in range(NC):
                # stage product rows into SBUF (add Sx on the f-side), then
                # multiply by the PE-broadcast reciprocal rows
                nc.vector.tensor_scalar(vf[:, CS[c]], prod[c][0:T, :], Sx[:], None, ALU.add)
                nc.scalar.activation(vb[:, CS[c]], prod[c][64:64 + T, :], AF.Identity)
                rrB = pgg.tile([T, 512], F32, name="rrB", tag="psG")
                nc.tensor.matmul(rrB[:], ones1[:], rr[:, CS[c]], start=True, stop=True)
                nc.vector.tensor_tensor(uf4[0:T, CS[c]], vf[:, CS[c]], rrB[:], ALU.mult)
                ccB = pgt.tile([T, 512], F32, name="ccB", tag="psGT")
                nc.tensor.matmul(ccB[:], ones1[:], cc[:, CS[c]], start=True, stop=True)
                nc.vector.tensor_tensor(ub4[0:T, CS[c]], vb[:, CS[c]], ccB[:], ALU.mult)

        # replicate uf/ub at partitions 32/64/96 for x_flat packing
        for g in range(1, 4):
            nc.sync.dma_start(out=uf4[32 * g:32 * g + T, :], in_=uf4[0:T, :])
            nc.sync.dma_start(out=ub4[32 * g:32 * g + T, :], in_=ub4[0:T, :])

        # =========== Stages D/E/F ===========
        with tc.tile_pool(name="pf", bufs=6, space="PSUM") as pf:
            zT = stile("zT", (128, 6, N))
            for k in range(6):
                ks = slice(k * 128, (k + 1) * 128)
                for c in range(NC):
                    g = (k * NC + c) % 4
                    b0 = 32 * g
                    ps = pf.tile([128, 512], F32, name="psF", tag="ps")
                    nc.tensor.matmul(ps[:], e1t4[b0:b0 + T, ks], xt4[b0:b0 + T, CS[c]],
                                     start=True, stop=False, tile_position=(b0, 0))
                    nc.tensor.matmul(ps[:], e2a4[b0:b0 + T, ks], uf4[b0:b0 + T, CS[c]],
                                     start=False, stop=False, tile_position=(b0, 0))
                    nc.tensor.matmul(ps[:], e2b4[b0:b0 + T, ks], ub4[b0:b0 + T, CS[c]],
                                     start=False, stop=True, tile_position=(b0, 0))
                    if (k * NC + c) % 2 == 0:
                        nc.scalar.activation(zT[:, k, CS[c]], ps[:], AF.Relu)
                    else:
                        nc.vector.tensor_scalar(zT[:, k, CS[c]], ps[:], 0.0, None, ALU.max)

            h1 = stile("h1", (HID2, N))
            for c in range(NC):
                ps = pf.tile([HID2, 512], F32, name="psH1", tag="ps")
                for k in range(6):
                    nc.tensor.matmul(ps[:], ew1[:, k, :], zT[:, k, CS[c]],
                                     start=(k == 0), stop=(k == 5))
                if c % 2 == 0:
                    nc.scalar.activation(h1[:, CS[c]], ps[:], AF.Relu, bias=eb1[:])
                else:
                    nc.vector.tensor_scalar(h1[:, CS[c]], ps[:], eb1[:], 0.0, ALU.add, ALU.max)

            h2 = stile("h2", (HID2, N))
            for c in range(NC):
                ps = pf.tile([HID2, 512], F32, name="psH2", tag="ps")
                nc.tensor.matmul(ps[:], ew2[:], h1[:, CS[c]], start=True, stop=True)
                if c % 2 == 1:
                    nc.scalar.activation(h2[:, CS[c]], ps[:], AF.Relu, bias=eb2[:])
                else:
                    nc.vector.tensor_scalar(h2[:, CS[c]], ps[:], eb2[:], 0.0, ALU.add, ALU.max)

            xe = stile("xe", (H, N))
            for c in range(NC):
                ps = pf.tile([H, 512], F32, name="psXe", tag="ps")
                nc.tensor.matmul(ps[:], ew3[:], h2[:, CS[c]], start=True, stop=False)
                for k in range(6):
                    nc.tensor.matmul(ps[:], eproj[:, k, :], zT[:, k, CS[c]],
                                     start=False, stop=(k == 5))
                if c % 2 == 0:
                    nc.scalar.activation(xe[:, CS[c]], ps[:], AF.Identity, bias=ebe[:])
                else:
                    nc.vector.tensor_scalar(xe[:, CS[c]], ps[:], ebe[:], None, ALU.add)

            g1 = stile("g1", (HID2, N))
            for c in range(NC):
                ps = pf.tile([HID2, 512], F32, name="psG1", tag="ps")
                nc.tensor.matmul(ps[:], dw1[:], xe[:, CS[c]], start=True, stop=True)
                if c % 2 == 1:
                    nc.scalar.activation(g1[:, CS[c]], ps[:], AF.Relu, bias=db1[:])
                else:
                    nc.vector.tensor_scalar(g1[:, CS[c]], ps[:], db1[:], 0.0, ALU.add, ALU.max)

            g2 = stile("g2", (HID2, N))
            for c in range(NC):
                ps = pf.tile([HID2, 512], F32, name="psG2", tag="ps")
                nc.tensor.matmul(ps[:], dw2[:], g1[:, CS[c]], start=True, stop=True)
                if c % 2 == 0:
                    nc.scalar.activation(g2[:, CS[c]], ps[:], AF.Relu, bias=db2[:])
                else:
                    nc.vector.tensor_scalar(g2[:, CS[c]], ps[:], db2[:], 0.0, ALU.add, ALU.max)

            od = stile("od", (TOUT, N), F32)
            for c in range(NC):
                ps = pf.tile([TOUT, 512], F32, name="psOd", tag="ps")
                nc.tensor.matmul(ps[:], dw3[:], g2[:, CS[c]], start=True, stop=False)
                nc.tensor.matmul(ps[:], dproj[:], xe[:, CS[c]], start=False, stop=True)
                if c % 2 == 1:
                    nc.scalar.activation(od[:, CS[c]], ps[:], AF.Identity, bias=dbd[:])
                else:
                    nc.vector.tensor_scalar(od[:, CS[c]], ps[:], dbd[:], None, ALU.add)

            nc.sync.dma_start(out=d["out"].ap(), in_=od[:])


def _host_weights(inputs):
    f32 = np.float32
    W1 = np.asarray(inputs["W1"], f32)[0]
    W2 = np.asarray(inputs["W2"], f32)[0]
    e1t = np.zeros((T, TH), f32)
    e2a = np.zeros((T, TH), f32)
    e2b = np.zeros((T, TH), f32)
    for t in range(T):
        e1t[t, t * H:(t + 1) * H] = W1
        e2a[t, t * H:(t + 1) * H] = 0.9 * W2   # K_HOPS * ALPHA
        e2b[t, t * H:(t + 1) * H] = 2.1 * W2   # K_HOPS * (1 - ALPHA)
    g = np.asarray(inputs["enc_bn_g"], f32); be = np.asarray(inputs["enc_bn_b"], f32)
    m = np.asarray(inputs["enc_bn_m"], f32); v = np.asarray(inputs["enc_bn_v"], f32)
    esc = g / np.sqrt(v + 1e-5)
    ew3 = np.asarray(inputs["enc_w3"], f32) * esc[None, :]
    eproj = np.asarray(inputs["enc_proj"], f32) * esc[None, :]
    ebe = np.asarray(inputs["enc_b3"], f32) * esc + (be - m * esc)
    g = np.asarray(inputs["dec_bn_g"], f32); bd = np.asarray(inputs["dec_bn_b"], f32)
    m = np.asarray(inputs["dec_bn_m"], f32); v = np.asarray(inputs["dec_bn_v"], f32)
    dsc = g / np.sqrt(v + 1e-5)
    dw3 = np.asarray(inputs["dec_w3"], f32) * dsc[None, :]
    dproj = np.asarray(inputs["dec_proj"], f32) * dsc[None, :]
    dbd = np.asarray(inputs["dec_b3"], f32) * dsc + (bd - m * dsc)
    col = lambda a: np.ascontiguousarray(np.asarray(a, f32).reshape(-1, 1))
    return {
        "e1t": e1t, "e2a": e2a, "e2b": e2b,
        "ew1": np.asarray(inputs["enc_w1"], f32), "ew2": np.asarray(inputs["enc_w2"], f32),
        "ew3": ew3, "eproj": eproj,
        "dw1": np.asarray(inputs["dec_w1"], f32), "dw2": np.asarray(inputs["dec_w2"], f32),
        "dw3": dw3, "dproj": dproj,
        "eb1": col(inputs["enc_b1"]), "eb2": col(inputs["enc_b2"]), "ebe": col(ebe),
        "db1": col(inputs["dec_b1"]), "db2": col(inputs["dec_b2"]), "dbd": col(dbd),
        "i12": np.eye(T, dtype=f32),
    }


def make_in_maps(inputs):
    wmap = _host_weights(inputs)
    x = np.asarray(inputs["x"], np.float32)
    in_maps = []
    for b in range(B):
        m = dict(wmap)
        m["x"] = np.ascontiguousarray(x[b, :, :, 0])
        in_maps.append(m)
    return in_maps


def kernel(**inputs) -> np.ndarray:
    from concourse.bass_utils import run_bass_kernel_spmd

    if "nc" not in _cache:
        _cache["nc"] = _build_nc()
    nc = _cache["nc"]

    in_maps = make_in_maps(inputs)
    trace = bool(int(os.environ.get("KERNEL_TRACE", "0")))
    res = run_bass_kernel_spmd(nc, in_maps, core_ids=list(range(B)), trace=trace)
    _cache["last_result"] = res
    out = np.stack([np.asarray(res.results[b]["out"]) for b in range(B)], axis=0)
    return out[..., None].astype(np.float32)


# revision 12
# speedup vs baseline: 67.4109x; 67.4109x over previous
"""Trainium2 Bass kernel for AdaBiDiff GNN message passing.

Per-core computation (data parallel over batch B=8, one batch element per core):
  xt (12,1536) -> softmax over t -> p, logp (t-major)
  kl[i,j] = rowterm[i] - sum_t p[i,t] logp[j,t];  A = (kl < 0.5)
  u_fwd = (A @ xt.T) / rowsum(A);  u_bwd = (A.T @ xt.T) / colsum(A)
  x_flat[n, t*64+h] = relu(xt[t,n] W1[h] + (0.9 u_fwd + 2.1 u_bwd)[n,t] W2[h])
  two MLP blocks (BN folded into weights on host) -> out (12,1536) per core.

Implementation notes:
  - augmented-G: phat=[p;0..;rowterm-0.5], lhat=[logp;0..;-1] at partitions
    0..11 and 32 (32-alignment), so Ghat = 0.5-kl and A = (Ghat > 0).
  - phat/lhat duplicated at partitions 64..96 so the two Ghat orientations
    run row-packed (tile_position (0,0) vs (64,0)) concurrently on the PE.
  - A-orientation compare on DVE (is_gt -> 0/1); AT-orientation on ScalarE
    (Sign -> -1/0/1), with the sign-affine correction folded into the
    u_fwd scaling: yA=(yA'+Sx)/2, rs=(rs'+N)/2 -> uf=(yA'+Sx)/(rs'+N).
  - ones column in the transposed-x stationary produces row/col sums free.
  - both product accumulators share PSUM banks (partitions 0-32 and 64-96
    of the same tiles) -> 3 banks, letting Ghat tiles double-buffer.
  - x_flat build 4-way row-packed (inputs replicated at partitions
    0/32/64/96).
  - matmul dtype float32r (1 col/cycle); A/AT tiles and xtT in bf16.
"""

import os
import numpy as np

import concourse.bass as bass
import concourse.bacc as bacc
import concourse.tile as tile
import concourse.mybir as mybir

F32 = mybir.dt.float32
F32R = mybir.dt.float32r
BF16 = mybir.dt.bfloat16
AF = mybir.ActivationFunctionType
ALU = mybir.AluOpType

B, T, N, H, TH, HID2, TOUT = 8, 12, 1536, 64, 768, 128, 12
NT = N // 128
NC = N // 512
AUG = 32

_cache = {}


def _build_nc():
    nc = bacc.Bacc("TRN2", target_bir_lowering=False, debug=False)
    d = {}

    def dp(name, shape, dt=F32R, out=False):
        d[name] = nc.declare_dram_parameter(name, list(shape), dt, isOutput=out)

    dp("x", (T, N))
    dp("e1t", (T, TH)); dp("e2a", (T, TH)); dp("e2b", (T, TH))
    dp("ew1", (TH, HID2)); dp("ew2", (HID2, HID2)); dp("ew3", (HID2, H)); dp("eproj", (TH, H))
    dp("dw1", (H, HID2)); dp("dw2", (HID2, HID2)); dp("dw3", (HID2, TOUT)); dp("dproj", (H, TOUT))
    dp("eb1", (HID2, 1), F32); dp("eb2", (HID2, 1), F32); dp("ebe", (H, 1), F32)
    dp("db1", (HID2, 1), F32); dp("db2", (HID2, 1), F32); dp("dbd", (TOUT, 1), F32)
    dp("i12", (T, T))
    dp("out", (T, N), F32, out=True)

    with tile.TileContext(nc) as tc:
        _kernel_body(tc, d)
    nc.compile()
    return nc


def _kernel_body(tc, d):
    nc = tc.nc
    CS = [slice(c * 512, (c + 1) * 512) for c in range(NC)]

    with tc.tile_pool(name="w", bufs=1) as w, tc.tile_pool(name="sb", bufs=1) as sb:

        def wload(name, shape, dt=F32R, src=None):
            t = w.tile(list(shape), dt, name=name, tag=name)
            nc.sync.dma_start(out=t[:], in_=src if src is not None else d[name].ap())
            return t

        def stile(name, shape, dt=F32R):
            return sb.tile(list(shape), dt, name=name, tag=name)

        # ---- inputs / weights; x and x_flat operands replicated at 4 offsets ----
        xt4 = stile("xt4", (128, N))
        e1t4 = stile("e1t4", (128, TH))
        e2a4 = stile("e2a4", (128, TH))
        e2b4 = stile("e2b4", (128, TH))
        for g in range(4):
            nc.sync.dma_start(out=xt4[32 * g:32 * g + T, :], in_=d["x"].ap())
            nc.sync.dma_start(out=e1t4[32 * g:32 * g + T, :], in_=d["e1t"].ap())
            nc.sync.dma_start(out=e2a4[32 * g:32 * g + T, :], in_=d["e2a"].ap())
            nc.sync.dma_start(out=e2b4[32 * g:32 * g + T, :], in_=d["e2b"].ap())
        xt = xt4[0:T, :]

        ew1 = wload("ew1", (128, 6, HID2), src=d["ew1"].ap().rearrange("(a p) m -> p a m", p=128))
        eproj = wload("eproj", (128, 6, H), src=d["eproj"].ap().rearrange("(a p) m -> p a m", p=128))
        ew2 = wload("ew2", (HID2, HID2))
        ew3 = wload("ew3", (HID2, H))
        dw1 = wload("dw1", (H, HID2))
        dw2 = wload("dw2", (HID2, HID2))
        dw3 = wload("dw3", (HID2, TOUT))
        dproj = wload("dproj", (H, TOUT))
        eb1 = wload("eb1", (HID2, 1), F32)
        eb2 = wload("eb2", (HID2, 1), F32)
        ebe = wload("ebe", (H, 1), F32)
        db1 = wload("db1", (HID2, 1), F32)
        db2 = wload("db2", (HID2, 1), F32)
        dbd = wload("dbd", (TOUT, 1), F32)
        i12 = wload("i12", (T, T))

        ones12 = w.tile([T, 1], F32R, name="ones12", tag="ones12")
        nc.vector.memset(ones12[:].bitcast(F32), 1.0)
        ones1 = w.tile([1, T], F32R, name="ones1", tag="ones1")
        nc.vector.memset(ones1[:].bitcast(F32), 1.0)
        nhalf = w.tile([1, 1], F32, name="nhalf", tag="nhalf")
        nc.vector.memset(nhalf[:], -0.5)
        # prewarm the exp/ln activation-table load under the input DMAs
        warm = w.tile([1, 1], F32, name="warm", tag="warm")
        nc.vector.memset(warm[:], 0.0)
        nc.scalar.activation(warm[:], warm[:], AF.Exp)

        # =========== Stage A: softmax chain (t-major) ===========
        phat = stile("phat", (97, N))
        lhat = stile("lhat", (97, N))
        xtT = stile("xtT", (128, NT, AUG + 1), BF16)
        Sx = stile("Sx", (T, 1), F32)

        nc.gpsimd.memset(phat[0:AUG + 1, :].bitcast(F32), 0.0)
        nc.gpsimd.memset(lhat[0:AUG + 1, :].bitcast(F32), 0.0)
        nc.gpsimd.memset(lhat[AUG:AUG + 1, :].bitcast(F32), -1.0)
        nc.gpsimd.memset(xtT[:], 0.0)

        nc.vector.tensor_reduce(Sx[:], xt, mybir.AxisListType.X, ALU.add)

        with tc.tile_pool(name="pa", bufs=2, space="PSUM") as pa, \
             tc.tile_pool(name="pat", bufs=1, space="PSUM") as pat:
            ex = stile("ex", (T, N))
            nc.scalar.activation(ex[:], xt, AF.Exp)

            ls = stile("ls", (1, N))
            for c in range(NC):
                psA = pa.tile([1, 512], F32, name="psA", tag="pa")
                nc.tensor.matmul(psA[:], ones12[:], ex[:, CS[c]], start=True, stop=True)
                nc.scalar.activation(ls[:, CS[c]], psA[:], AF.Ln)

            for c in range(NC):
                lsB = pa.tile([T, 512], F32, name="lsB", tag="pa")
                nc.tensor.matmul(lsB[:], ones1[:], ls[:, CS[c]], start=True, stop=True)
                nc.vector.tensor_tensor(lhat[0:T, CS[c]], xt[:, CS[c]], lsB[:], ALU.subtract)

            nc.scalar.activation(phat[0:T, :], lhat[0:T, :], AF.Exp)

            q = stile("q", (T, N))
            nc.vector.tensor_tensor(q[:], phat[0:T, :], lhat[0:T, :], ALU.mult)
            for c in range(NC):
                psR = pa.tile([1, 512], F32, name="psR", tag="pa")
                nc.tensor.matmul(psR[:], ones12[:], q[:, CS[c]], start=True, stop=True)
                nc.scalar.activation(phat[AUG:AUG + 1, CS[c]], psR[:], AF.Identity, bias=nhalf[:])

            # transposed x with ones column (bf16): xtT[p, j, t] = xt[t, 128j+p]
            psT = pat.tile([128, NT, T], F32, name="psT", tag="psT")
            for j in range(NT):
                nc.tensor.matmul(psT[:, j, :], xt[:, j * 128:(j + 1) * 128], i12[:],
                                 start=True, stop=True)
            nc.vector.tensor_copy(xtT[:, :, 0:T], psT[:])
            nc.vector.memset(xtT[:, :, AUG:AUG + 1], 1.0)

        # duplicate phat/lhat at partitions 64..96 for PE row-packing
        nc.sync.dma_start(out=phat[64:97, :], in_=phat[0:AUG + 1, :])
        nc.sync.dma_start(out=lhat[64:97, :], in_=lhat[0:AUG + 1, :])

        # =========== Stage B: Ghat, adjacency, products ===========
        uf4 = stile("uf4", (128, N))
        ub4 = stile("ub4", (128, N))

        with tc.tile_pool(name="pp", bufs=1, space="PSUM") as pp, \
             tc.tile_pool(name="pgg", bufs=2, space="PSUM") as pgg, \
             tc.tile_pool(name="pgt", bufs=2, space="PSUM") as pgt, \
             tc.tile_pool(name="ab", bufs=3) as ab:

            # shared-bank product accumulators: rows 0..32 = [yA';rs'] (sign),
            # rows 64..96 = [yAT;cs] (0/1)
            prod = [pp.tile([128, 512], F32, name=f"prod{c}", tag=f"prod{c}") for c in range(NC)]

            for i in range(NT):
                Ai = ab.tile([128, N], BF16, name="Ai", tag="Ai")
                ATi = ab.tile([128, N], BF16, name="ATi", tag="ATi")
                isl = slice(i * 128, (i + 1) * 128)
                for c in range(NC):
                    psG = pgg.tile([128, 512], F32, name="psG", tag="psG")
                    nc.tensor.matmul(psG[:], phat[0:AUG + 1, isl], lhat[0:AUG + 1, CS[c]],
                                     start=True, stop=True, tile_position=(0, 0))
                    nc.vector.tensor_scalar(Ai[:, CS[c]], psG[:], 0.0, None, ALU.is_gt)
                    psGT = pgt.tile([128, 512], F32, name="psGT", tag="psGT")
                    nc.tensor.matmul(psGT[:], lhat[64:97, isl], phat[64:97, CS[c]],
                                     start=True, stop=True, tile_position=(64, 0))
                    nc.scalar.sign(ATi[:, CS[c]], psGT[:])
                for c in range(NC):
                    nc.tensor.matmul(prod[c][0:AUG + 1, :], xtT[:, i, :], ATi[:, CS[c]],
                                     start=(i == 0), stop=(i == NT - 1),
                                     skip_group_check=True, tile_position=(0, 0))
                    nc.tensor.matmul(prod[c][64:97, :], xtT[:, i, :], Ai[:, CS[c]],
                                     start=(i == 0), stop=(i == NT - 1),
                                     skip_group_check=True, tile_position=(0, 64))

            # ===== Stage C: scale products =====
            # uf = (yA' + Sx) / (rs' + N)   [sign-corrected];  ub = yAT / cs
            rr = stile("rr", (1, N), F32R)
            cc = stile("cc", (1, N), F32R)
            tmpf = stile("tmpf", (1, N), F32)
            vf = stile("vf", (T, N), F32)
            vb = stile("vb", (T, N), F32)
            nN = w.tile([1, 1], F32, name="nN", tag="nN")
            nc.vector.memset(nN[:], float(N))
            with nc.allow_low_precision(reason="4-byte recips"):
                for c in range(NC):
                    nc.scalar.activation(tmpf[:, CS[c]], prod[c][AUG:AUG + 1, :],
                                         AF.Identity, bias=nN[:])
                    nc.vector.reciprocal(rr[:, CS[c]], tmpf[:, CS[c]])
                    nc.vector.reciprocal(cc[:, CS[c]], prod[c][96:97, :])
            for c in range(NC):
                # stage product rows into SBUF (add Sx on the f-side), then
                # multiply by the PE-broadcast reciprocal rows
                nc.scalar.activation(vf[:, CS[c]], prod[c][0:T, :], AF.Identity, bias=Sx[:])
                nc.scalar.activation(vb[:, CS[c]], prod[c][64:64 + T, :], AF.Identity)
                rrB = pgg.tile([T, 512], F32, name="rrB", tag="psG")
                nc.tensor.matmul(rrB[:], ones1[:], rr[:, CS[c]], start=True, stop=True)
                nc.vector.tensor_tensor(uf4[0:T, CS[c]], vf[:, CS[c]], rrB[:], ALU.mult)
                ccB = pgt.tile([T, 512], F32, name="ccB", tag="psGT")
                nc.tensor.matmul(ccB[:], ones1[:], cc[:, CS[c]], start=True, stop=True)
                nc.vector.tensor_tensor(ub4[0:T, CS[c]], vb[:, CS[c]], ccB[:], ALU.mult)

        # replicate uf/ub at partitions 32/64/96 for x_flat packing
        for g in range(1, 4):
            nc.sync.dma_start(out=uf4[32 * g:32 * g + T, :], in_=uf4[0:T, :])
            nc.sync.dma_start(out=ub4[32 * g:32 * g + T, :], in_=ub4[0:T, :])

        # =========== Stages D/E/F ===========
        with tc.tile_pool(name="pf", bufs=6, space="PSUM") as pf:
            zT = stile("zT", (128, 6, N))
            for k in range(6):
                ks = slice(k * 128, (k + 1) * 128)
                for c in range(NC):
                    g = (k * NC + c) % 4
                    b0 = 32 * g
                    ps = pf.tile([128, 512], F32, name="psF", tag="ps")
                    nc.tensor.matmul(ps[:], e1t4[b0:b0 + T, ks], xt4[b0:b0 + T, CS[c]],
                                     start=True, stop=False, tile_position=(b0, 0))
                    nc.tensor.matmul(ps[:], e2a4[b0:b0 + T, ks], uf4[b0:b0 + T, CS[c]],
                                     start=False, stop=False, tile_position=(b0, 0))
                    nc.tensor.matmul(ps[:], e2b4[b0:b0 + T, ks], ub4[b0:b0 + T, CS[c]],
                                     start=False, stop=True, tile_position=(b0, 0))
                    if (k * NC + c) % 2 == 0:
                        nc.scalar.activation(zT[:, k, CS[c]], ps[:], AF.Relu)
                    else:
                        nc.vector.tensor_scalar(zT[:, k, CS[c]], ps[:], 0.0, None, ALU.max)

            h1 = stile("h1", (HID2, N))
            for c in range(NC):
                ps = pf.tile([HID2, 512], F32, name="psH1", tag="ps")
                for k in range(6):
                    nc.tensor.matmul(ps[:], ew1[:, k, :], zT[:, k, CS[c]],
                                     start=(k == 0), stop=(k == 5))
                if c % 2 == 0:
                    nc.scalar.activation(h1[:, CS[c]], ps[:], AF.Relu, bias=eb1[:])
                else:
                    nc.vector.tensor_scalar(h1[:, CS[c]], ps[:], eb1[:], 0.0, ALU.add, ALU.max)

            h2 = stile("h2", (HID2, N))
            for c in range(NC):
                ps = pf.tile([HID2, 512], F32, name="psH2", tag="ps")
                nc.tensor.matmul(ps[:], ew2[:], h1[:, CS[c]], start=True, stop=True)
                if c % 2 == 1:
                    nc.scalar.activation(h2[:, CS[c]], ps[:], AF.Relu, bias=eb2[:])
                else:
                    nc.vector.tensor_scalar(h2[:, CS[c]], ps[:], eb2[:], 0.0, ALU.add, ALU.max)

            xe = stile("xe", (H, N))
            for c in range(NC):
                ps = pf.tile([H, 512], F32, name="psXe", tag="ps")
                nc.tensor.matmul(ps[:], ew3[:], h2[:, CS[c]], start=True, stop=False)
                for k in range(6):
                    nc.tensor.matmul(ps[:], eproj[:, k, :], zT[:, k, CS[c]],
                                     start=False, stop=(k == 5))
                if c % 2 == 0:
                    nc.scalar.activation(xe[:, CS[c]], ps[:], AF.Identity, bias=ebe[:])
                else:
                    nc.vector.tensor_scalar(xe[:, CS[c]], ps[:], ebe[:], None, ALU.add)

            g1 = stile("g1", (HID2, N))
            for c in range(NC):
                ps = pf.tile([HID2, 512], F32, name="psG1", tag="ps")
                nc.tensor.matmul(ps[:], dw1[:], xe[:, CS[c]], start=True, stop=True)
                if c % 2 == 1:
                    nc.scalar.activation(g1[:, CS[c]], ps[:], AF.Relu, bias=db1[:])
                else:
                    nc.vector.tensor_scalar(g1[:, CS[c]], ps[:], db1[:], 0.0, ALU.add, ALU.max)

            g2 = stile("g2", (HID2, N))
            for c in range(NC):
                ps = pf.tile([HID2, 512], F32, name="psG2", tag="ps")
                nc.tensor.matmul(ps[:], dw2[:], g1[:, CS[c]], start=True, stop=True)
                if c % 2 == 0:
                    nc.scalar.activation(g2[:, CS[c]], ps[:], AF.Relu, bias=db2[:])
                else:
                    nc.vector.tensor_scalar(g2[:, CS[c]], ps[:], db2[:], 0.0, ALU.add, ALU.max)

            od = stile("od", (TOUT, N), F32)
            for c in range(NC):
                ps = pf.tile([TOUT, 512], F32, name="psOd", tag="ps")
                nc.tensor.matmul(ps[:], dw3[:], g2[:, CS[c]], start=True, stop=False)
                nc.tensor.matmul(ps[:], dproj[:], xe[:, CS[c]], start=False, stop=True)
                if c % 2 == 1:
                    nc.scalar.activation(od[:, CS[c]], ps[:], AF.Identity, bias=dbd[:])
                else:
                    nc.vector.tensor_scalar(od[:, CS[c]], ps[:], dbd[:], None, ALU.add)

            nc.sync.dma_start(out=d["out"].ap(), in_=od[:])


def _host_weights(inputs):
    f32 = np.float32
    W1 = np.asarray(inputs["W1"], f32)[0]
    W2 = np.asarray(inputs["W2"], f32)[0]
    e1t = np.zeros((T, TH), f32)
    e2a = np.zeros((T, TH), f32)
    e2b = np.zeros((T, TH), f32)
    for t in range(T):
        e1t[t, t * H:(t + 1) * H] = W1
        e2a[t, t * H:(t + 1) * H] = 0.9 * W2   # K_HOPS * ALPHA
        e2b[t, t * H:(t + 1) * H] = 2.1 * W2   # K_HOPS * (1 - ALPHA)
    g = np.asarray(inputs["enc_bn_g"], f32); be = np.asarray(inputs["enc_bn_b"], f32)
    m = np.asarray(inputs["enc_bn_m"], f32); v = np.asarray(inputs["enc_bn_v"], f32)
    esc = g / np.sqrt(v + 1e-5)
    ew3 = np.asarray(inputs["enc_w3"], f32) * esc[None, :]
    eproj = np.asarray(inputs["enc_proj"], f32) * esc[None, :]
    ebe = np.asarray(inputs["enc_b3"], f32) * esc + (be - m * esc)
    g = np.asarray(inputs["dec_bn_g"], f32); bd = np.asarray(inputs["dec_bn_b"], f32)
    m = np.asarray(inputs["dec_bn_m"], f32); v = np.asarray(inputs["dec_bn_v"], f32)
    dsc = g / np.sqrt(v + 1e-5)
    dw3 = np.asarray(inputs["dec_w3"], f32) * dsc[None, :]
    dproj = np.asarray(inputs["dec_proj"], f32) * dsc[None, :]
    dbd = np.asarray(inputs["dec_b3"], f32) * dsc + (bd - m * dsc)
    col = lambda a: np.ascontiguousarray(np.asarray(a, f32).reshape(-1, 1))
    return {
        "e1t": e1t, "e2a": e2a, "e2b": e2b,
        "ew1": np.asarray(inputs["enc_w1"], f32), "ew2": np.asarray(inputs["enc_w2"], f32),
        "ew3": ew3, "eproj": eproj,
        "dw1": np.asarray(inputs["dec_w1"], f32), "dw2": np.asarray(inputs["dec_w2"], f32),
        "dw3": dw3, "dproj": dproj,
        "eb1": col(inputs["enc_b1"]), "eb2": col(inputs["enc_b2"]), "ebe": col(ebe),
        "db1": col(inputs["dec_b1"]), "db2": col(inputs["dec_b2"]), "dbd": col(dbd),
        "i12": np.eye(T, dtype=f32),
    }


def make_in_maps(inputs):
    wmap = _host_weights(inputs)
    x = np.asarray(inputs["x"], np.float32)
    in_maps = []
    for b in range(B):
        m = dict(wmap)
        m["x"] = np.ascontiguousarray(x[b, :, :, 0])
        in_maps.append(m)
    return in_maps


def kernel(**inputs) -> np.ndarray:
    from concourse.bass_utils import run_bass_kernel_spmd

    if "nc" not in _cache:
        _cache["nc"] = _build_nc()
    nc = _cache["nc"]

    in_maps = make_in_maps(inputs)
    trace = bool(int(os.environ.get("KERNEL_TRACE", "0")))
    res = run_bass_kernel_spmd(nc, in_maps, core_ids=list(range(B)), trace=trace)
    _cache["last_result"] = res
    out = np.stack([np.asarray(res.results[b]["out"]) for b in range(B)], axis=0)
    return out[..., None].astype(np.float32)
